# revision 30
# baseline (speedup 1.0000x reference)
"""GAT layer (gnn_message_passing) Bass kernel for 8 Trainium2 NeuronCores.

Row-sharded: core c computes output rows [c*R, (c+1)*R) of
    out = softmax(mask(leakyrelu(s_src[i]+s_dst[j]), adj)) @ (h @ W.T)

v4 design (on top of v3's transposed layout + bf16 PE traffic):
  - Factorized exp + row-invariance: softmax rows are invariant to a
    positive per-row scale, so instead of p = exp(leakyrelu(s_i+t_j)) =
    max(e^x, e^{0.2x}) the kernel aggregates Z = p / e^{0.2 s_i} =
    max(E_i*B_j, D_j) with E=e^{0.8 s_src}, B=e^{s_dst}, D=e^{0.2 s_dst}.
    The O(N^2) elementwise work is ONE dual-op tensor_scalar per chunk:
    (E_bcast mult B_j) max D_j -- B_j and D_j are per-partition f32
    scalar APs, so the op keeps DVE's fast mode. All O(N^2) ACT Exp
    volume (56us in v3) collapses to three O(N) vector exps.
  - Mask applied by the DMA engine with zero extra compute ops: walrus
    only allows accum_op=add on SWDGE, so the mask is an INTEGER add on
    the bf16 BIT PATTERN. madj holds {0, -12288} fp8e5 (-12288 = -96<<7,
    exact); the accum DMA targets the max() output bitcast to int16, so
    masked entries get their bf16 exponent dropped by 96 -> value scaled
    by 2^-96 ~ 0. Safe: X = max(U,V) >= e^-1, so the exponent field
    never borrows into the sign bit.
  - Row sums: v3's ones_col [128,1] stationary serialized all PSUM
    writes onto one partition (~600ns per 512-col matmul vs ~350 for the
    main matmul). v4 uses a group-indicator stationary G8 [128,8]
    (partition j -> group j//16) so writes spread across 8 partitions;
    the [8, R] partial sums collapse in the tail via PE transpose + DVE
    free-axis reduce.
  - Wh matmuls interleave with the attention stream (one merged loop,
    elementwise trails Wh by 4 chunks) so the PE never idles waiting on
    the mask DMA and HAM keeps the clock warm. The mask SWDGE for a
    group issues as soon as its last chunk's Z is emitted; the group's
    matmuls trail a full group behind.

Layout: transposed on device, [j (source node) on partitions, i (dest node)
on free]. p.T tiles feed the TensorEngine directly as moving operands for
outT += Wh[jc].T @ pT with zero on-chip transposes.
"""

import functools
import sys

sys.path.insert(0, "/opt/trn_rl_repo")

import numpy as np
import ml_dtypes

import bass_rust
import concourse.bass as bass
import concourse.mybir as mybir
import concourse.tile as tile
from concourse.masks import make_identity
from concourse.bass_utils import run_bass_kernel_spmd

F32 = mybir.dt.float32
BF16 = mybir.dt.bfloat16
FP8 = mybir.dt.float8e4
FP8E5 = mybir.dt.float8e5
I16 = mybir.dt.int16
AF = mybir.ActivationFunctionType
ALU = mybir.AluOpType

N_CORES = 8


def _patch_tail_drain():
    """This walrus build caps sync waits at 1 per instruction (2 for EVSEM),
    but Tile emits multi-wait instructions in two places: regular insts via
    assign_waits, and the tail drain. Split surplus waits onto same-engine
    wait-only NOPs placed immediately before (regular) / after (tail drain)
    the owning instruction."""
    from concourse.tile import ScopedClock, TileContext

    if getattr(TileContext, "_drain_patched", False):
        return

    _orig_loi = TileContext._lower_ordered_insts

    def _lower_ordered_insts(self, ordered):
        nc = self.nc
        ws_id = 0
        for bbname in list(ordered.keys()):
            insts = ordered[bbname]
            new = []
            for inst in insts:
                si = inst.sync_info
                if si is not None:
                    cap = 2 if isinstance(inst, mybir.InstEventSemaphore) else 1
                    waits = list(si.on_wait)
                    if len(waits) > cap:
                        extra, keep = waits[:-cap], waits[-cap:]
                        for w in extra:
                            nop = mybir.InstNoOp(
                                name=f"{inst.name}-ws{ws_id}", ins=[], outs=[]
                            )
                            ws_id += 1
                            nop.engine = inst.engine
                            nop.sync_info = bass_rust.SyncInfo(
                                on_wait=[w], on_update=[]
                            )
                            nc.register_instruction(nop, overwrite=True)
                            new.append(nop)
                        inst.sync_info = bass_rust.SyncInfo(
                            on_wait=keep, on_update=list(si.on_update)
                        )
                new.append(inst)
            ordered[bbname] = new
        return _orig_loi(self, ordered)

    TileContext._lower_ordered_insts = _lower_ordered_insts

    def _drain_and_barrier(self, tick_clock, wait_clock):
        drain_inst = self.nc.sync.drain()
        wait_clock.add_sem_waits(
            drain_inst.ins, ScopedClock({None: tick_clock.global_clock})
        )
        si = drain_inst.ins.sync_info
        if si is not None and len(si.on_wait) > 1:
            waits = list(si.on_wait)
            drain_inst.ins.sync_info = bass_rust.SyncInfo(
                on_wait=[waits[0]], on_update=list(si.on_update)
            )
            for w in waits[1:]:
                nop = self.nc.sync.nop(nofuse=True)
                nop.ins.sync_info = bass_rust.SyncInfo(on_wait=[w], on_update=[])
        self.nc.all_engine_barrier()
        assert self.sems is not None
        popped = self.nc._tile_sem_poison_stack.pop()
        assert popped is self._sem_poison
        self.nc.clear_and_free_semaphores(list(self.sems.allocated().values()))
        self.nc.all_engine_barrier()

    TileContext._drain_and_barrier = _drain_and_barrier
    TileContext._drain_patched = True


def build_gat_nc(N=8192, R=1024, FIN=256, FOUT=128):
    """Build the per-core Bass program (transposed layout). All cores run the
    same program on different data slices."""
    import os

    # bisection knobs (default = fastest path)
    swdge_split = int(os.environ.get("GAT_SWDGE_SPLIT", "2"))  # chunks per accum DMA
    no_dma_mask = bool(int(os.environ.get("GAT_NO_DMA_MASK", "0")))
    tt_mask = os.environ.get("GAT_TT_MASK", "most")  # even|none|all: which groups use the DVE int16-add mask path
    _patch_tail_drain()

    P = 128
    FK = FIN // P          # fin chunks (contraction for Wh)
    NCH = N // P           # 128-row j-chunks over all N source nodes
    RB = R // P            # 128-wide i-subblocks per core
    SEG = 512 if R % 512 == 0 else R
    NSEG = R // SEG
    EB = 4 if NCH % 4 == 0 else 1   # chunks per mask-DMA group
    WB = 2 if NCH % 2 == 0 else 1   # Wh chunks per PSUM tile

    nc = bass.Bass()
    hT_t = nc.dram_tensor("hT", [FIN, N], BF16, kind="ExternalInput")
    hTo_t = nc.dram_tensor("hT_own", [FIN, R], BF16, kind="ExternalInput")
    # mask, fp8 {0,1}, pre-arranged so group G lives at rows [G*128,(G+1)*128)
    # with the EB chunks of the group concatenated along the free dim.
    madj_t = nc.dram_tensor("madj8", [(NCH // EB) * P, EB * R], FP8E5, kind="ExternalInput")
    madj16_t = nc.dram_tensor("madj16", [(NCH // EB) * P, EB * R], I16, kind="ExternalInput")
    w_t = nc.dram_tensor("W", [FOUT, FIN], F32, kind="ExternalInput")
    wT_t = nc.dram_tensor("WT", [FIN, FOUT], BF16, kind="ExternalInput")
    a_t = nc.dram_tensor("a", [2 * FOUT, 1], F32, kind="ExternalInput")
    out_t = nc.dram_tensor("out_blk", [R, FOUT], F32, kind="ExternalOutput")

    with tile.TileContext(nc) as tc:
        with tc.tile_pool(name="persist", bufs=1) as persist:
            ident = persist.tile([P, P], F32)
            make_identity(nc, ident)
            ones_row = persist.tile([1, P], BF16)
            nc.vector.memset(ones_row, 1.0)
            # group-indicator stationary: G8[j, g] = 1 iff j//16 == g,
            # built as the intersection of two affine half-planes.
            g8 = persist.tile([P, 8], BF16)
            nc.gpsimd.memset(g8, 1.0)
            nc.gpsimd.affine_select(
                out=g8, in_=g8, compare_op=ALU.is_ge, fill=0.0,
                base=0, channel_multiplier=1, pattern=[[-16, 8]],
            )  # keep where j - 16g >= 0
            nc.gpsimd.affine_select(
                out=g8, in_=g8, compare_op=ALU.is_ge, fill=0.0,
                base=15, channel_multiplier=-1, pattern=[[16, 8]],
            )  # keep where 15 - j + 16g >= 0
            HSP = 8 if N % 8 == 0 else 1                 # hT slabs (dep granularity)
            hT_sb = [
                persist.tile([P, FK, N // HSP], BF16, name=f"hT{s}")
                for s in range(HSP)
            ]                                            # h.T, fin on partitions
            hTo_sb = persist.tile([P, FK, R], BF16)      # own rows of h.T
            whs_sb = persist.tile([P, NCH, FOUT], BF16)  # Wh, j on partitions
            bcol = persist.tile([P, NCH, 1], F32)        # B_j = exp(s_dst)
            dcol = persist.tile([P, NCH, 1], F32)        # D_j = exp(0.2 s_dst)
            scol = persist.tile([P, NCH, 1], F32)        # raw s_dst staging
            ebcast = persist.tile([P, R], BF16)          # E_i = exp(0.8 s_src), bcast
            rhs_aug = persist.tile([P, FK, FOUT + 1], BF16)  # [W.T | w_dst] per fin chunk
            erow_sb = persist.tile([1, R], BF16)         # exp(0.8 s_src) row
            wsrc_sb = persist.tile([P, FK], BF16)        # w_src per fin chunk

            # startup DMAs: spread dispatch across engine sequencers (each
            # HWDGE dispatch costs ~600ns of sequencer time). W/a/wT go
            # first: Wh(0) blocks on the rhs_aug chain, so its inputs must
            # clear the scalar queue before the bulkier hTo loads.
            for k in range(FK):
                nc.sync.dma_start(
                    out=rhs_aug[:, k, 0:FOUT], in_=wT_t[k * P : (k + 1) * P, :]
                )
            HPC = N // HSP
            for s0 in range(HSP):
                c0 = s0 * HPC
                for k in range(FK):
                    nc.sync.dma_start(
                        out=hT_sb[s0][:, k, :],
                        in_=hT_t[k * P : (k + 1) * P, c0 : c0 + HPC],
                    )

            # warmers: touch the SWDGE ring and the ACT Exp table early so
            # their one-time setup cost (~12us / ~1.3us) is off the critical
            # path; also start the first TT-mask tile load immediately.
            warm_sb = persist.tile([1, 2], BF16)
            nc.vector.memset(warm_sb, 0.0)
            nc.gpsimd.dma_start(
                out=warm_sb[0:1, 0:1].bitcast(I16),
                in_=madj_t[0:1, 0:1],
                accum_op=ALU.add,
            )
            warm_exp = persist.tile([1, 2], BF16)
            nc.scalar.activation(out=warm_exp, in_=warm_sb, func=AF.Exp, bias=0.0)

            # ---------------- prologue: w_src/w_dst, A/C rows ----------------
            with (
                tc.tile_pool(name="pro", bufs=1) as pro,
                tc.tile_pool(name="pro_ps", bufs=1, space="PSUM") as pro_ps,
            ):
                w_sb = pro.tile([P, FIN], F32)
                nc.scalar.dma_start(out=w_sb, in_=w_t[:, :])
                acol = pro.tile([P, 2], F32)
                nc.scalar.dma_start(out=acol[:, 0:1], in_=a_t[0:FOUT, :])        # a_src
                nc.scalar.dma_start(out=acol[:, 1:2], in_=a_t[FOUT : 2 * FOUT, :])  # a_dst
                for k in range(FK):
                    nc.scalar.dma_start(
                        out=hTo_sb[:, k, :], in_=hTo_t[k * P : (k + 1) * P, :]
                    )

                for k in range(FK):
                    wchunk = w_sb[:, k * P : (k + 1) * P]
                    pw = pro_ps.tile([P, 2], F32, tag="wv")
                    nc.tensor.matmul(pw[:, 0:1], wchunk, acol[:, 1:2], start=True, stop=True)
                    nc.tensor.matmul(pw[:, 1:2], wchunk, acol[:, 0:1], start=True, stop=True)
                    nc.vector.tensor_copy(out=rhs_aug[:, k, FOUT : FOUT + 1], in_=pw[:, 0:1])
                    nc.vector.tensor_copy(out=wsrc_sb[:, k : k + 1], in_=pw[:, 1:2])

                # s_src directly as a [1, R] row: stationary = w_src column,
                # moving = hTo. (1-partition PSUM writes serialize, but 4
                # matmuls beat v3's col-matmuls + 8 fp32 transposes.)
                srow_ps = pro_ps.tile([1, R], F32, tag="srow")
                SSEG = 512 if R % 512 == 0 else R
                for s in range(R // SSEG):
                    for k in range(FK):
                        nc.tensor.matmul(
                            srow_ps[:, s * SSEG : (s + 1) * SSEG],
                            wsrc_sb[:, k : k + 1],
                            hTo_sb[:, k, s * SSEG : (s + 1) * SSEG],
                            start=(k == 0),
                            stop=(k == FK - 1),
                        )
                nc.scalar.activation(
                    out=erow_sb, in_=srow_ps, func=AF.Exp, bias=0.0, scale=0.8
                )

            # ------------- main: Wh phase, then attention -------------
            with (
                tc.tile_pool(name="whp", bufs=2, space="PSUM") as whp,
                tc.tile_pool(name="e1p", bufs=6) as e1p,
                tc.tile_pool(name="mtlp", bufs=5) as mtlp,
                tc.tile_pool(name="xbp", bufs=8) as xbp,
                tc.tile_pool(name="sm", bufs=2) as sm,
                tc.tile_pool(name="osb", bufs=2) as osb,
                tc.tile_pool(name="out_ps", bufs=1, space="PSUM") as out_ps,
                tc.tile_pool(name="tr_ps", bufs=2, space="PSUM") as tr_ps,
            ):
                psum_outT = [
                    out_ps.tile([P, SEG], F32, tag=f"poT{s}", name=f"poT{s}")
                    for s in range(NSEG)
                ]
                psum_sums = [
                    out_ps.tile([P, SEG], F32, tag=f"psm{s}", name=f"psm{s}")
                    for s in range(NSEG)
                ]
                xb = None
                pending = []
                pending_tt = []
                MMDEPTH = 4  # matmuls trail this many groups behind the mask
                LAG = 2 * EB  # elementwise trails the Wh stream by two groups

                def emit_wh(c2):
                    wh_ps = whp.tile([P, WB, FOUT + 1], F32, tag="wh", name="wh_ps")
                    CPS = NCH // HSP  # chunks per hT slab
                    for i in range(WB):
                        c = c2 * WB + i
                        for k in range(FK):
                            nc.tensor.matmul(
                                wh_ps[:, i, :],
                                hT_sb[c // CPS][
                                    :, k, (c % CPS) * P : (c % CPS + 1) * P
                                ],
                                rhs_aug[:, k, :],
                                start=(k == 0),
                                stop=(k == FK - 1),
                            )
                    nc.scalar.activation(
                        out=whs_sb[:, c2 * WB : (c2 + 1) * WB, :],
                        in_=wh_ps[:, :, 0:FOUT],
                        func=AF.Copy,
                        bias=0.0,
                    )
                    # tiny DVE read frees the PSUM tile ~500ns sooner; the
                    # two ACT exps then read SBUF off the PE critical path
                    nc.vector.tensor_copy(
                        out=scol[:, c2 * WB : (c2 + 1) * WB, :],
                        in_=wh_ps[:, :, FOUT : FOUT + 1],
                    )
                    nc.scalar.activation(
                        out=bcol[:, c2 * WB : (c2 + 1) * WB, :],
                        in_=scol[:, c2 * WB : (c2 + 1) * WB, :],
                        func=AF.Exp,
                        bias=0.0,
                    )
                    nc.scalar.activation(
                        out=dcol[:, c2 * WB : (c2 + 1) * WB, :],
                        in_=scol[:, c2 * WB : (c2 + 1) * WB, :],
                        func=AF.Exp,
                        bias=0.0,
                        scale=0.2,
                    )

                def is_tt_grp(grp):
                    if tt_mask == "all":
                        return True
                    if tt_mask == "none":
                        return False
                    if tt_mask == "most":
                        # 3/4 on the DVE int16-add path, 1/4 on SWDGE: DVE is
                        # the pacing engine, but 4 light SWDGE groups stay well
                        # under the DMA engines' RMW rate
                        return grp % 4 != 1
                    return grp % 2 == 0

                def emit_bcast():
                    # broadcast erow down the partitions, using the (not yet
                    # accumulating) psum_outT banks as scratch
                    for s in range(NSEG):
                        nc.tensor.matmul(
                            psum_outT[s],
                            ones_row,
                            erow_sb[:, s * SEG : (s + 1) * SEG],
                            start=True,
                            stop=True,
                        )
                        nc.vector.tensor_copy(
                            out=ebcast[:, s * SEG : (s + 1) * SEG],
                            in_=psum_outT[s],
                        )

                mtl_tiles = {}

                def prefetch_mask16(grp):
                    mtl = mtlp.tile([P, EB, R], I16, tag="m16", name="mtl16")
                    nc.sync.dma_start(
                        out=mtl, in_=madj16_t[grp * P : (grp + 1) * P, :]
                    )
                    mtl_tiles[grp] = mtl

                def mask_slice(jc, xbt):
                    """SWDGE bit-mask for the sw-chunk slice ending at chunk
                    jc; issued as soon as the slice's Z chunks are emitted so
                    the DMA has maximum slack before the matmuls need it."""
                    grp = jc // EB
                    sw = swdge_split if swdge_split else EB
                    gg = (jc % EB) - (sw - 1)
                    nc.gpsimd.dma_start(
                        out=xbt[:, gg : gg + sw, :].bitcast(I16),
                        in_=madj_t[
                            grp * P : (grp + 1) * P,
                            gg * R : (gg + sw) * R,
                        ],
                        accum_op=ALU.add,
                    )

                def mm_group(jc_last, xbt):
                    """outT/sums matmuls for the masked group ending at
                    jc_last. Emitted one group late (software pipeline)."""
                    jc0 = jc_last - (EB - 1)
                    for s in range(NSEG):
                        for gg in range(EB):
                            jcc = jc0 + gg
                            q = jcc % 4
                            nc.tensor.matmul(
                                psum_sums[s][32 * q : 32 * q + 8, :],
                                g8,
                                xbt[:, gg, s * SEG : (s + 1) * SEG],
                                start=(jcc < 4),
                                stop=(jcc >= NCH - 4),
                                tile_position=(0, 32 * q),
                                skip_group_check=True,
                            )
                    for gg in range(EB):
                        jcc = jc0 + gg
                        for s in range(NSEG):
                            nc.tensor.matmul(
                                psum_outT[s],
                                whs_sb[:, jcc, :],
                                xbt[:, gg, s * SEG : (s + 1) * SEG],
                                start=(jcc == 0),
                                stop=(jcc == NCH - 1),
                            )

                def emit_elem(jc):
                    nonlocal xb, pending
                    g = jc % EB
                    grp = jc // EB
                    if g == 0:
                        xb = xbp.tile([P, EB, R], BF16, tag="xb", name="xb")
                        if is_tt_grp(grp) and grp not in mtl_tiles:
                            prefetch_mask16(grp)
                        if (
                            grp + 2 < NCH // EB
                            and is_tt_grp(grp + 2)
                            and grp + 2 not in mtl_tiles
                        ):
                            prefetch_mask16(grp + 2)
                    # Z = (E_i * B_j) max D_j in one dual-op tensor_scalar
                    nc.vector.tensor_scalar(
                        out=xb[:, g, :],
                        in0=ebcast,
                        scalar1=bcol[:, jc, :],
                        scalar2=dcol[:, jc, :],
                        op0=ALU.mult,
                        op1=ALU.max,
                    )
                    sw = swdge_split if swdge_split else EB
                    if not is_tt_grp(grp) and (jc % EB) % sw == sw - 1:
                        mask_slice(jc, xb)
                    if g != EB - 1:
                        return
                    # delay the TT mask one group so the Z stream (which
                    # feeds every downstream engine) never queues behind a
                    # ~2us mask op on DVE
                    if pending_tt:
                        pgrp, pxb = pending_tt.pop(0)
                        mtl = mtl_tiles.pop(pgrp)
                        nc.vector.tensor_tensor(
                            out=pxb.bitcast(I16),
                            in0=pxb.bitcast(I16),
                            in1=mtl,
                            op=ALU.add,
                        )
                    if is_tt_grp(grp):
                        pending_tt.append((grp, xb))
                    if len(pending) >= MMDEPTH:
                        mm_group(*pending.pop(0))
                    pending.append((jc, xb))

                for grp0 in (0, 1):
                    if grp0 < NCH // EB and is_tt_grp(grp0):
                        prefetch_mask16(grp0)
                for c2 in range(NCH // WB):
                    emit_wh(c2)
                    if c2 == 2:
                        emit_bcast()
                    for i in range(WB):
                        jc = c2 * WB + i - LAG
                        if jc >= 0:
                            emit_elem(jc)
                for jc in range(NCH - LAG, NCH):
                    emit_elem(jc)
                while pending_tt:
                    pgrp, pxb = pending_tt.pop(0)
                    mtl = mtl_tiles.pop(pgrp)
                    nc.vector.tensor_tensor(
                        out=pxb.bitcast(I16),
                        in0=pxb.bitcast(I16),
                        in1=mtl,
                        op=ALU.add,
                    )
                while pending:
                    mm_group(*pending.pop(0))

                # tail: collapse [8, R] group sums -> per-partition recip,
                # transpose out.T blocks, scale, store.
                # evacuate the 4 written quadrant stripes (zero the rest),
                # then fold 32 stripe-rows -> 8 group-rows with one matmul per
                # segment (f32 re-written into the freed psum rows 0:8).
                sums32_sb = sm.tile([P, R], BF16, tag="s32", name="sums32_sb")
                nc.vector.memset(sums32_sb, 0.0)
                for s in range(NSEG):
                    for q in range(4):
                        nc.scalar.activation(
                            out=sums32_sb[
                                32 * q : 32 * q + 8, s * SEG : (s + 1) * SEG
                            ],
                            in_=psum_sums[s][32 * q : 32 * q + 8, :],
                            func=AF.Copy,
                            bias=0.0,
                        )
                fold8 = sm.tile([P, 8], BF16, tag="f8w", name="fold8")
                nc.vector.memset(fold8, 0.0)
                for q in range(4):
                    nc.vector.tensor_copy(
                        out=fold8[32 * q : 32 * q + 8, :], in_=ident[0:8, 0:8]
                    )
                sums8_sb = sm.tile([8, R], F32, tag="s8", name="sums8_sb")
                for s in range(NSEG):
                    nc.tensor.matmul(
                        psum_sums[s][0:8, :],
                        fold8,
                        sums32_sb[:, s * SEG : (s + 1) * SEG],
                        start=True,
                        stop=True,
                    )
                    nc.vector.tensor_copy(
                        out=sums8_sb[:, s * SEG : (s + 1) * SEG],
                        in_=psum_sums[s][0:8, :],
                    )
                rsums_ps = tr_ps.tile([P, P], F32, tag="tr", name="rsums_ps")
                for b in range(RB):
                    nc.tensor.transpose(
                        rsums_ps[:, b * 8 : (b + 1) * 8],
                        sums8_sb[:, b * P : (b + 1) * P],
                        ident[0:8, 0:8],
                    )
                rsum_col = sm.tile([P, RB], F32, tag="rsc", name="rsum_col")
                for b in range(RB):
                    nc.vector.tensor_reduce(
                        out=rsum_col[:, b : b + 1],
                        in_=rsums_ps[:, b * 8 : (b + 1) * 8],
                        axis=mybir.AxisListType.X,
                        op=ALU.add,
                    )
                recip_col = sm.tile([P, RB], F32, tag="rcc", name="recip_col")
                nc.vector.reciprocal(recip_col, rsum_col)
                outT_sb = sm.tile([P, R], F32, tag="oT", name="outT_sb")
                BPS = SEG // P  # i-blocks per segment
                for b in range(RB):
                    nc.vector.tensor_copy(
                        out=outT_sb[:, b * P : (b + 1) * P],
                        in_=psum_outT[b // BPS][
                            :, (b % BPS) * P : (b % BPS + 1) * P
                        ],
                    )
                    tr = tr_ps.tile([P, P], F32, tag="tr", name="tr")
                    nc.tensor.transpose(
                        tr, outT_sb[:, b * P : (b + 1) * P], ident
                    )
                    out_sb = osb.tile([P, FOUT], F32, tag="ob", name="out_sb")
                    nc.scalar.activation(
                        out=out_sb,
                        in_=tr,
                        func=AF.Copy,
                        bias=0.0,
                        scale=recip_col[:, b : b + 1],
                    )
                    nc.sync.dma_start(out=out_t[b * P : (b + 1) * P, :], in_=out_sb)

    return nc


@functools.lru_cache(maxsize=2)
def _compiled(N, R, FIN, FOUT):
    return build_gat_nc(N=N, R=R, FIN=FIN, FOUT=FOUT)


def run_gat(h, adj, W, a, trace=False, tmpdir=None):
    BF = ml_dtypes.bfloat16
    E4 = ml_dtypes.float8_e4m3
    h = np.asarray(h, dtype=np.float32)
    adj = np.asarray(adj, dtype=np.int32)
    N, FIN = h.shape
    FOUT = np.asarray(W).shape[0]
    R = N // N_CORES
    P = 128
    NCH = N // P
    EB = 4 if NCH % 4 == 0 else 1
    nc = _compiled(N, R, FIN, FOUT)

    hT_bf = np.ascontiguousarray(h.T.astype(BF))
    W32 = np.ascontiguousarray(np.asarray(W, dtype=np.float32))
    WT_bf = np.ascontiguousarray(W32.T.astype(BF))
    a32 = np.ascontiguousarray(np.asarray(a, dtype=np.float32).reshape(2 * FOUT, 1))
    # exponent-drop mask: adj==0 -> int-add -12288 onto the bf16 bits
    # (value *= 2^-96 ~ 0); adj==1 -> +0 (unchanged). fp8_e5m2 holds
    # -12288 = -1.5*2^13 exactly; the int16 variant feeds the DVE TT path.
    E5 = ml_dtypes.float8_e5m2
    lut = np.array([-12288.0, 0.0], dtype=E5)
    lut16 = np.array([-12288, 0], dtype=np.int16)

    in_maps = []
    for c in range(N_CORES):
        sl = slice(c * R, (c + 1) * R)
        adjT = adj[sl].T
        # group-major layout: [NCH//EB, EB, P, R] -> [NCH//EB, P, EB, R]
        def gmajor(m):
            return np.ascontiguousarray(
                m.reshape(NCH // EB, EB, P, R)
                .transpose(0, 2, 1, 3)
                .reshape((NCH // EB) * P, EB * R)
            )

        m8 = gmajor(lut[adjT])
        m16 = gmajor(lut16[adjT])
        in_maps.append(
            {
                "hT": hT_bf,
                "hT_own": np.ascontiguousarray(h[sl].T.astype(BF)),
                "madj8": m8,
                "madj16": m16,
                "W": W32,
                "WT": WT_bf,
                "a": a32,
            }
        )
    res = run_bass_kernel_spmd(
        nc, in_maps, core_ids=list(range(N_CORES)), trace=trace, tmpdir=tmpdir
    )
    out = np.concatenate([r["out_blk"] for r in res.results], axis=0)
    return out, res


def kernel(h, adj, W, a):
    out, _ = run_gat(np.asarray(h), np.asarray(adj), np.asarray(W), np.asarray(a))
    return out.astype(np.float32)
